# revision 12
# baseline (speedup 1.0000x reference)
# BinaryLinear on 8 Trainium2 NeuronCores.
#
# y = x @ sign(W)^T + bias for x [8192, 4096] f32, W [4096, 4096] f32.
#
# Sharding: data-parallel over the 8192 tokens (1024 per core), per the
# problem's sharding hint. Each core runs one [K=4096, M=1024] x [K=4096,
# N=4096] matmul: stationary operand = x^T shard in bf16, moving operand =
# sign(W)^T in fp8e4m3 (+-1 is exact in fp8, and the PE accepts mixed
# bf16-stationary x fp8-moving at full bf16 rate), f32 PSUM accumulation.
# x -> bf16 rounding is the only approximation (~1.7e-3 relative output err).
#
# Execution goes through bass2jax/PJRT (axon): one jitted shard_map over the
# 8-core mesh. The donated output backing buffer is created on-device so no
# zero-filled bytes cross the host->device link.

import numpy as np
import ml_dtypes

N_TOKENS = 8192
IN_F = 4096
OUT_F = 4096
N_CORES = 8
TOK_SHARD = N_TOKENS // N_CORES

_C = {}


OUT_DT = "float16"  # device-side output dtype (upcast to f32 on host).
# f16 keeps D2H small; rounding f32 PSUM results to f16 adds ~3e-4 relative
# error rms on top of the ~1.7e-3 from x->bf16 — negligible.


ORIENT = "x_stat"  # "x_stat": x^T is stationary, y [tok, out] out.
#                    "w_stat": sign(W)^T is stationary, y^T [out, tok] out.


def _build_nc(
    out_dt=None,
    max_k_tile=512,
    kxm_bufs=None,
    kxn_bufs=None,
    max_tile=512,
    free_dim=512,
    repeats=1,
    orient=None,
):
    import concourse.mybir as mybir
    import concourse.tile as tile
    from concourse import bacc
    from concourse.kernels.tile_matmul import matmul_tile_kernel

    out_dt = out_dt or OUT_DT
    orient = orient or ORIENT
    nc = bacc.Bacc("TRN2", target_bir_lowering=False, debug=False)
    x_t = nc.dram_tensor(
        "x_t", [IN_F, TOK_SHARD], mybir.dt.bfloat16, kind="ExternalInput"
    ).ap()
    w_t = nc.dram_tensor(
        "w_t", [IN_F, OUT_F], mybir.dt.float8e4, kind="ExternalInput"
    ).ap()
    out_shape = [TOK_SHARD, OUT_F] if orient == "x_stat" else [OUT_F, TOK_SHARD]
    y = nc.dram_tensor(
        "y", out_shape, getattr(mybir.dt, out_dt), kind="ExternalOutput"
    ).ap()
    with tile.TileContext(nc) as tc:
        import contextlib

        with contextlib.ExitStack() as es:
            kxm_pool = (
                es.enter_context(tc.tile_pool(name="kxm_pool", bufs=kxm_bufs))
                if kxm_bufs
                else None
            )
            kxn_pool = (
                es.enter_context(tc.tile_pool(name="kxn_pool", bufs=kxn_bufs))
                if kxn_bufs
                else None
            )
            kxm_ap, kxn_ap = (x_t, w_t) if orient == "x_stat" else (w_t, x_t)
            for _ in range(repeats):
                matmul_tile_kernel(
                    tc,
                    kxm_ap,
                    kxn_ap,
                    y,
                    kxm_pool=kxm_pool,
                    kxn_pool=kxn_pool,
                    MAX_K_TILE_SIZE=max_k_tile,
                    MAX_TILE_SIZE=max_tile,
                    MATMUL_FREE_DIM=free_dim,
                )
    nc.compile()
    return nc


def _get_nc():
    if "nc" not in _C:
        _C["nc"] = _build_nc()
    return _C["nc"]


def _get_runner():
    """Compile the 8-core jitted executable once; returns (fn, zeros_fn)."""
    if "runner" in _C:
        return _C["runner"]
    import jax
    import jax.numpy as jnp
    from jax.sharding import Mesh, NamedSharding, PartitionSpec

    import inspect

    try:
        from jax.experimental.shard_map import shard_map
    except ImportError:
        from jax import shard_map
    _rep_kw = (
        {"check_rep": False}
        if "check_rep" in inspect.signature(shard_map).parameters
        else {"check_vma": False}
    )
    import concourse.mybir as mybir
    from concourse import bass2jax
    from concourse.bass2jax import _bass_exec_p, install_neuronx_cc_hook

    nc = _get_nc()
    install_neuronx_cc_hook()

    partition_name = nc.partition_id_tensor.name if nc.partition_id_tensor else None
    in_names, out_names, out_avals = [], [], []
    for alloc in nc.m.functions[0].allocations:
        if not isinstance(alloc, mybir.MemoryLocationSet):
            continue
        name = alloc.memorylocations[0].name
        if alloc.kind == "ExternalInput":
            if name != partition_name:
                in_names.append(name)
        elif alloc.kind == "ExternalOutput":
            out_names.append(name)
            out_avals.append(
                jax.core.ShapedArray(
                    tuple(alloc.tensor_shape), mybir.dt.np(alloc.dtype)
                )
            )
    assert in_names == ["x_t", "w_t"] and out_names == ["y"], (in_names, out_names)
    all_in_names = list(in_names) + list(out_names)
    if partition_name is not None:
        all_in_names.append(partition_name)

    def _body(*args):
        operands = list(args)
        if partition_name is not None:
            operands.append(bass2jax.partition_id_tensor())
        outs = _bass_exec_p.bind(
            *operands,
            out_avals=tuple(out_avals),
            in_names=tuple(all_in_names),
            out_names=tuple(out_names),
            lowering_input_output_aliases=(),
            sim_require_finite=True,
            sim_require_nnan=True,
            nc=nc,
        )
        return tuple(outs)

    devices = jax.devices()[:N_CORES]
    mesh = Mesh(np.asarray(devices), ("core",))
    sharding = NamedSharding(mesh, PartitionSpec("core"))
    in_specs = (PartitionSpec("core"),) * 3  # x_t, w_t, y-backing
    out_specs = (PartitionSpec("core"),)
    fn = jax.jit(
        shard_map(_body, mesh=mesh, in_specs=in_specs, out_specs=out_specs,
                  **_rep_kw),
        donate_argnums=(2,),
        keep_unused=True,
    )
    out_np_dt = out_avals[0].dtype
    zeros_fn = jax.jit(
        lambda: jnp.zeros((N_TOKENS, OUT_F), out_np_dt),
        out_shardings=sharding,
    )
    _C["runner"] = (fn, zeros_fn, sharding, jax)
    return _C["runner"]


def _host_prep(x, weight):
    """sign/transpose/cast/shard on the host (cheap vs the matmul)."""
    xt = np.ascontiguousarray(np.asarray(x).T).astype(ml_dtypes.bfloat16)
    # global stacked layout for shard_map: axis0 = concat of per-core shards
    xg = np.concatenate(
        [xt[:, c * TOK_SHARD : (c + 1) * TOK_SHARD] for c in range(N_CORES)],
        axis=0,
    )
    wt = np.ascontiguousarray(np.sign(np.asarray(weight)).T).astype(
        ml_dtypes.float8_e4m3
    )
    wg = np.concatenate([wt] * N_CORES, axis=0)
    return xg, wg


def _run_spmd_fallback(x, weight):
    """Conservative path through bass_utils.run_bass_kernel_spmd (same
    underlying bass2jax/PJRT execution; pays extra host->device bytes for the
    zero-filled output backing buffers)."""
    from concourse.bass_utils import run_bass_kernel_spmd

    nc = _get_nc()
    xt = np.ascontiguousarray(np.asarray(x).T).astype(ml_dtypes.bfloat16)
    wt = np.ascontiguousarray(np.sign(np.asarray(weight)).T).astype(
        ml_dtypes.float8_e4m3
    )
    in_maps = [
        {"x_t": np.ascontiguousarray(xt[:, c * TOK_SHARD : (c + 1) * TOK_SHARD]),
         "w_t": wt}
        for c in range(N_CORES)
    ]
    res = run_bass_kernel_spmd(nc, in_maps, core_ids=list(range(N_CORES)))
    return np.concatenate([r["y"] for r in res.results], axis=0)


def kernel(x, weight, bias):
    try:
        fn, zeros_fn, sharding, jax = _get_runner()
        xg, wg = _host_prep(x, weight)
        xd = jax.device_put(xg, sharding)
        wd = jax.device_put(wg, sharding)
        y_backing = zeros_fn()
        (yd,) = fn(xd, wd, y_backing)
        # global [8192, 4096], token order preserved
        y = np.asarray(yd)
    except Exception:
        y = _run_spmd_fallback(x, weight)
    # upcast + bias on host
    y = y.astype(np.float32)
    y += np.asarray(bias, dtype=np.float32)[None, :]
    return y
